# revision 34
# baseline (speedup 1.0000x reference)
"""Trainium2 Bass kernel for batched cross-attention (B=8, Lq=1024, Lk=2048, D=1024).

Sharding: pure data-parallel over the batch dim — each of the 8 NeuronCores
computes full attention for one batch element. Weights are replicated.

Per core:
  Q = q_b @ Wq^T + bq          [1024, 1024]
  K = x_b @ Wk^T + bk          [2048, 1024]
  V = x_b @ Wv^T + bv          [2048, 1024]
  S = Q @ K^T / sqrt(D)        [1024, 2048]
  A = softmax(S, axis=-1)      [1024, 2048]  (output 2)
  O = A @ V                    [1024, 1024]  (output 1)

All matmuls run as float32r (full-rate fp32 on the PE at moving-dim>=256).
The TensorEngine contracts over the partition dim, so every matmul operand
needs the contraction dim (d, e or k) on partitions. x^T, q^T and the W^T
matrices are produced on the HOST (input marshalling in kernel()) and DMA'd
directly in d-major layout — only the two unavoidable on-device transposes
remain (Q -> Q^T after the projection, and exp(S) -> exp(S)^T for the
weighted matmul), done on the PE via identity matmuls with 4 transposes
sharing one PSUM bank per batched PSUM->SBUF copy.

Phase A reads x^T once and computes BOTH K^T (kept in SBUF) and V. V is
bounced through DRAM: its write happens in phase A, its read at the start of
DMA-slack phase B, keeping phase A under the ~200GB/s DMA ceiling.

The weighted matmul consumes the *unnormalized* exp(S) transpose; the softmax
1/rowsum is folded into the PSUM->SBUF copy of the weighted output as a
per-partition scale, so the PE never waits on the softmax reduction.
"""

import sys

import numpy as np

if "/opt/trn_rl_repo" not in sys.path:
    sys.path.insert(0, "/opt/trn_rl_repo")

P = 128
D = 1024  # model dim
LQ = 1024  # query length
LK = 2048  # key length
B = 8  # batch == number of cores
DC = D // P  # 8 chunks of the contraction (d) dim
EC = D // P  # 8 chunks of the output-feature (e) dim
KC = LK // P  # 16 chunks of the key dim
KB = 512  # k-block width for phase A streaming
NKB = LK // KB  # 4
NQB = LQ // P  # 8 query blocks
SCALE = 1.0 / 32.0  # 1/sqrt(D)

_CACHE = {}


def build_nc(use_bias=True):
    import concourse.mybir as mybir
    import concourse.tile as tile
    from concourse import bacc
    from concourse.masks import make_identity

    F32 = mybir.dt.float32
    F32R = mybir.dt.float32r
    Exp = mybir.ActivationFunctionType.Exp
    Ident = mybir.ActivationFunctionType.Identity
    Copy = mybir.ActivationFunctionType.Copy
    AX = mybir.AxisListType.X

    nc = bacc.Bacc(
        "TRN2",
        target_bir_lowering=False,
        debug=False,
        enable_asserts=False,
        num_devices=B,
    )

    # host-transposed (d-major) inputs
    xT_d = nc.dram_tensor("xT", [D, LK], F32R, kind="ExternalInput").ap()
    qT_d = nc.dram_tensor("qT", [D, LQ], F32R, kind="ExternalInput").ap()
    wqT_d = nc.dram_tensor("WqT", [D, D], F32R, kind="ExternalInput").ap()
    wkT_d = nc.dram_tensor("WkT", [D, D], F32R, kind="ExternalInput").ap()
    wvT_d = nc.dram_tensor("WvT", [D, D], F32R, kind="ExternalInput").ap()
    if use_bias:
        bq_d = nc.dram_tensor("bq", [D], F32, kind="ExternalInput").ap()
        bk_d = nc.dram_tensor("bk", [D], F32, kind="ExternalInput").ap()
        bv_d = nc.dram_tensor("bv", [D], F32, kind="ExternalInput").ap()
    out_d = nc.dram_tensor("weighted", [LQ, D], F32, kind="ExternalOutput").ap()
    attn_d = nc.dram_tensor("attention", [LQ, LK], F32, kind="ExternalOutput").ap()
    # V bounce, e-half-major so phase B can fetch each half contiguously:
    # V_dram[eh, p, ko, j] = V[ko*128 + p, eh*512 + j]
    v_dram = nc.dram_tensor("V_scratch", [2, P, KC, 512], F32R).ap()

    # d-major DRAM views: [(c p), n] -> [p, c, n]
    xT_v = xT_d.rearrange("(c p) n -> p c n", p=P)
    qT_v = qT_d.rearrange("(c p) n -> p c n", p=P)
    wqT_v = wqT_d.rearrange("(c p) n -> p c n", p=P)
    wkT_v = wkT_d.rearrange("(c p) n -> p c n", p=P)
    wvT_v = wvT_d.rearrange("(c p) n -> p c n", p=P)

    with tile.TileContext(nc) as tc:
        with (
            tc.tile_pool(name="const", bufs=1) as cpool,
            tc.tile_pool(name="persist", bufs=1) as persist,
            tc.tile_pool(name="psumT4", bufs=3, space="PSUM") as psumT4,
            tc.tile_pool(name="psumMM", bufs=5, space="PSUM") as psumMM,
        ):
            # phase-B working pools live at top level so the A->B handoff has
            # no pool-boundary space dependency
            _pb_ctxs = [
                tc.tile_pool(name="pb_exp", bufs=1),
            ]
            (expp,) = [c.__enter__() for c in _pb_ctxs]

            ident = cpool.tile([P, P], F32, tag="ident")
            make_identity(nc, ident[:])

            if use_bias:
                # per-partition bias layouts: b[(eo p)] -> [p, eo]
                bk_sb = cpool.tile([P, EC], F32, tag="bk")
                nc.sync.dma_start(bk_sb[:], bk_d.rearrange("(o p) -> p o", p=P))
                bq_sb = cpool.tile([P, EC], F32, tag="bq")
                nc.sync.dma_start(bq_sb[:], bq_d.rearrange("(o p) -> p o", p=P))

                # ones-column trick operands for the V bias (free-dim bias):
                # onescol[p, m] = (p == 0); bvpad[0, :] = bv, others zero.
                # psum += onescol.T @ bvpad broadcasts bv to all partitions.
                onescol = cpool.tile([P, P], F32R, tag="onescol")
                bvpad = cpool.tile([P, D], F32R, tag="bvpad")

            # K^T [e, k] and Q^T stay resident from phase A through phase B.
            KT = persist.tile([P, EC, LK], F32R, tag="KT")  # KT[p, eo, k]
            # Q^T in two 4-q-block groups: QTg[g][p, ec, (qb%4)*128 + i]
            QTg = [
                persist.tile([P, EC, 512], F32R, tag=f"QT{g}", name=f"QTg{g}")
                for g in range(2)
            ]

            def transpose_batch(src, src_off, dst4, dst_c0, nblk):
                """PE-transpose `nblk` (<=4) contiguous [P, P] blocks of `src`
                starting at free-offset `src_off`, into dst4[:, dst_c0+j, :]
                via one shared PSUM bank and a single batched copy."""
                pst = psumT4.tile([P, 4, P], F32, tag="pT4")
                for j in range(nblk):
                    nc.tensor.transpose(
                        pst[:, j, :],
                        src[:, src_off + j * P : src_off + (j + 1) * P],
                        ident[:],
                    )
                nc.any.tensor_copy(
                    dst4[:, dst_c0 : dst_c0 + nblk, :], pst[:, :nblk, :]
                )

            # ------------- Phase A: K^T (SBUF) + V (DRAM) from one x^T pass --
            with (
                tc.tile_pool(name="pa_wT", bufs=1) as wTp,
                tc.tile_pool(name="pa_xT", bufs=2) as xTp,
                tc.tile_pool(name="pa_vstg", bufs=2) as vstgp,
                tc.tile_pool(name="pa_stg", bufs=1) as stgp,
            ):
                # PE warmup: dummy fp32 matmuls on the identity so the HAM
                # clock-gate reaches 8/8 while the first DMAs stream.
                wps = psumMM.tile([P, 512], F32, tag="pMM")
                for w in range(8):
                    nc.tensor.matmul(
                        wps[:, 0:P],
                        ident[:],
                        ident[:],
                        start=(w == 0),
                        stop=(w == 7),
                    )

                if use_bias:
                    # f32r tiles can't be memset/DMA'd directly: stage in f32
                    # and let ACT copies do the f32->f32r rounding.
                    stage = stgp.tile([P, D], F32, tag="stg")
                    nc.gpsimd.memset(stage[:], 0.0)
                    nc.gpsimd.memset(stage[0:1, 0:P], 1.0)
                    nc.scalar.copy(onescol[:], stage[:, 0:P])
                    bvstage = stgp.tile([P, D], F32, tag="stg")
                    nc.gpsimd.memset(bvstage[:], 0.0)
                    nc.sync.dma_start(
                        bvstage[0:1, :], bv_d.rearrange("(a d) -> a d", a=1)
                    )
                    nc.scalar.copy(bvpad[:], bvstage[:])

                WkT = wTp.tile([P, DC, D], F32R, tag="WkT")
                WvT = wTp.tile([P, DC, D], F32R, tag="WvT")

                def xt_block(kb, per_dc=False):
                    xT = xTp.tile([P, DC, KB], F32R, tag="xT")
                    if per_dc:
                        # interleave per-dc chunks of WkT and x^T(kb0) so the
                        # first KT matmuls start after ~1MB of DMA, not 6MB
                        for dc in range(DC):
                            nc.sync.dma_start(
                                WkT[:, dc, :], wkT_v[:, dc, :]
                            )
                            nc.sync.dma_start(
                                xT[:, dc, :],
                                xT_v[:, dc, kb * KB : (kb + 1) * KB],
                            )
                        for dc in range(DC):
                            nc.sync.dma_start(
                                WvT[:, dc, :], wvT_v[:, dc, :]
                            )
                    else:
                        for h in range(2):
                            nc.sync.dma_start(
                                xT[:, h * (DC // 2) : (h + 1) * (DC // 2), :],
                                xT_v[
                                    :,
                                    h * (DC // 2) : (h + 1) * (DC // 2),
                                    kb * KB : (kb + 1) * KB,
                                ],
                            )
                    return xT

                def kt_block(kb, xT):
                    # KT[:, ec, kb] += sum_dc WkT[:, dc, ec].T @ xT[:, dc, :]
                    for ec in range(EC):
                        ps = psumMM.tile([P, 512], F32, tag="pMM")
                        for dc in range(DC):
                            nc.tensor.matmul(
                                ps[:, :KB],
                                WkT[:, dc, ec * P : (ec + 1) * P],
                                xT[:, dc, :],
                                start=(dc == 0),
                                stop=(dc == DC - 1),
                            )
                        if use_bias:
                            # copy + per-partition bias bk[e]
                            nc.scalar.activation(
                                KT[:, ec, kb * KB : (kb + 1) * KB],
                                ps[:, :KB],
                                Ident,
                                bias=bk_sb[:, ec : ec + 1],
                            )
                        else:
                            nc.any.tensor_copy(
                                KT[:, ec, kb * KB : (kb + 1) * KB], ps[:, :KB]
                            )

                def v_block(kb, xT):
                    # V[kb*4+k4] = sum_dc xT[:,dc,k4].T @ WvT[:,dc,:] (+ bv),
                    # staged out to DRAM for phase B
                    for k4 in range(KB // P):
                        ko = kb * (KB // P) + k4
                        for eh in range(2):
                            ps = psumMM.tile([P, 512], F32, tag="pMM")
                            if use_bias:
                                nc.tensor.matmul(
                                    ps[:],
                                    onescol[:],
                                    bvpad[:, eh * 512 : (eh + 1) * 512],
                                    start=True,
                                    stop=False,
                                )
                            for dc in range(DC):
                                nc.tensor.matmul(
                                    ps[:],
                                    xT[:, dc, k4 * P : (k4 + 1) * P],
                                    WvT[:, dc, eh * 512 : (eh + 1) * 512],
                                    start=(dc == 0 and not use_bias),
                                    stop=(dc == DC - 1),
                                )
                            vstg = vstgp.tile([P, 512], F32R, tag="vstg")
                            nc.any.tensor_copy(vstg[:], ps[:])
                            nc.sync.dma_start(v_dram[eh, :, ko, :], vstg[:])

                for kb in range(NKB):
                    xT = xt_block(kb, per_dc=(kb == 0))
                    kt_block(kb, xT)
                    v_block(kb, xT)

                # ---- tail of phase A: build Q^T directly (no Q transpose)
                # WqT reuses WkT's slot (same tag); qT reuses the xT slots.
                WqT = wTp.tile([P, DC, D], F32R, tag="WkT")
                nc.sync.dma_start(WqT[:, 0 : DC // 2, :], wqT_v[:, 0 : DC // 2, :])
                nc.sync.dma_start(WqT[:, DC // 2 :, :], wqT_v[:, DC // 2 :, :])
                for g in range(2):
                    qT4 = xTp.tile([P, DC, 512], F32R, tag="xT")
                    nc.sync.dma_start(
                        qT4[:], qT_v[:, :, g * 512 : (g + 1) * 512]
                    )
                    # QT[e, q] = sum_dc WqT[:, dc, e].T @ qT4[:, dc, :] (+ bq)
                    for ec in range(EC):
                        ps = psumMM.tile([P, 512], F32, tag="pMM")
                        for dc in range(DC):
                            nc.tensor.matmul(
                                ps[:],
                                WqT[:, dc, ec * P : (ec + 1) * P],
                                qT4[:, dc, :],
                                start=(dc == 0),
                                stop=(dc == DC - 1),
                            )
                        if use_bias:
                            nc.scalar.activation(
                                QTg[g][:, ec, :],
                                ps[:],
                                Ident,
                                bias=bq_sb[:, ec : ec + 1],
                            )
                        else:
                            nc.any.tensor_copy(QTg[g][:, ec, :], ps[:])

            # ------------- Phase B: per-q-block attention --------------------
            with (
                tc.tile_pool(name="pb_v", bufs=1) as vp,
                tc.tile_pool(name="pb_small", bufs=1) as smallp,
                tc.tile_pool(name="pb_out", bufs=1) as outp,
                tc.tile_pool(name="pb_attnT", bufs=1) as attnTp,
            ):
                # fetch V from the bounce buffer, e-half 0 first (the first
                # weighted matmul needs only half 0)
                V = vp.tile([P, KC, D], F32R, tag="V")  # V[p, ko, e]
                for eh in range(2):
                    nc.sync.dma_start(
                        V[:, :, eh * 512 : (eh + 1) * 512], v_dram[eh]
                    )

                for qb in range(NQB):
                    qs = qb * P
                    QT = QTg[qb // 4]
                    qo = (qb % 4) * P

                    # scores (psum) -> exp + row-sum, in chunks of 512
                    exp_sb = expp.tile([P, LK], F32, tag="exp")
                    sums4 = smallp.tile([P, 4], F32, tag="sums4")
                    for kq in range(LK // 512):
                        ps = psumMM.tile([P, 512], F32, tag="pMM")
                        for ec in range(EC):
                            nc.tensor.matmul(
                                ps[:],
                                QT[:, ec, qo : qo + P],
                                KT[:, ec, kq * 512 : (kq + 1) * 512],
                                start=(ec == 0),
                                stop=(ec == EC - 1),
                            )
                        nc.scalar.activation(
                            exp_sb[:, kq * 512 : (kq + 1) * 512],
                            ps[:],
                            Exp,
                            scale=SCALE,
                            accum_out=sums4[:, kq : kq + 1],
                        )

                    # transpose UNNORMALIZED exp -> attnT[k-part, q]; the
                    # 1/rowsum is applied on the weighted output instead,
                    # so the PE never waits on the softmax reduction.
                    attnT = attnTp.tile([P, KC, P], F32R, tag="attnT")
                    for g in range(KC // 4):
                        transpose_batch(exp_sb, g * 4 * P, attnT, g * 4, 4)

                    sumk = smallp.tile([P, 1], F32, tag="sumk")
                    nc.vector.reduce_sum(sumk[:], sums4[:], axis=AX)
                    rsum = smallp.tile([P, 1], F32, tag="rsum")
                    nc.vector.reciprocal(rsum[:], sumk[:])
                    # normalize in place (after the transposes read it),
                    # write attention out
                    nc.vector.tensor_scalar_mul(exp_sb[:], exp_sb[:], rsum[:])
                    nc.sync.dma_start(attn_d[qs : qs + P, :], exp_sb[:])

                    # weighted [q, e] = (sum_kc attnT[:, kc].T @ V) * rsum
                    wout = outp.tile([P, D], F32, tag="wout")
                    for eh in range(2):
                        ps = psumMM.tile([P, 512], F32, tag="pMM")
                        for kc in range(KC):
                            nc.tensor.matmul(
                                ps[:],
                                attnT[:, kc, :],
                                V[:, kc, eh * 512 : (eh + 1) * 512],
                                start=(kc == 0),
                                stop=(kc == KC - 1),
                            )
                        nc.scalar.activation(
                            wout[:, eh * 512 : (eh + 1) * 512],
                            ps[:],
                            Copy,
                            scale=rsum[:],
                        )
                    nc.sync.dma_start(out_d[qs : qs + P, :], wout[:])

            for c in reversed(_pb_ctxs):
                c.__exit__(None, None, None)

    nc.compile()
    return nc


def _get_nc(use_bias=True):
    key = ("nc", use_bias)
    if key not in _CACHE:
        _CACHE[key] = build_nc(use_bias=use_bias)
    return _CACHE[key]


def make_in_maps(inputs, use_bias):
    """Host-side input marshalling: shard over batch and pre-transpose to the
    d-major layouts the kernel consumes."""
    wqT = np.ascontiguousarray(np.asarray(inputs["Wq"], dtype=np.float32).T)
    wkT = np.ascontiguousarray(np.asarray(inputs["Wk"], dtype=np.float32).T)
    wvT = np.ascontiguousarray(np.asarray(inputs["Wv"], dtype=np.float32).T)
    in_maps = []
    for b in range(B):
        m = {
            "xT": np.ascontiguousarray(
                np.asarray(inputs["x"][b], dtype=np.float32).T
            ),
            "qT": np.ascontiguousarray(
                np.asarray(inputs["q"][b], dtype=np.float32).T
            ),
            "WqT": wqT,
            "WkT": wkT,
            "WvT": wvT,
        }
        if use_bias:
            m["bq"] = np.asarray(inputs["bq"], dtype=np.float32)
            m["bk"] = np.asarray(inputs["bk"], dtype=np.float32)
            m["bv"] = np.asarray(inputs["bv"], dtype=np.float32)
        in_maps.append(m)
    return in_maps


def kernel(**inputs):
    from concourse.bass_utils import run_bass_kernel_spmd

    use_bias = any(
        np.any(np.asarray(inputs[k])) for k in ("bq", "bk", "bv")
    )
    nc = _get_nc(use_bias=use_bias)
    in_maps = make_in_maps(inputs, use_bias)
    res = run_bass_kernel_spmd(nc, in_maps, core_ids=list(range(B)))
    weighted = np.stack([res.results[b]["weighted"] for b in range(B)])
    attention = np.stack([res.results[b]["attention"] for b in range(B)])
    return weighted, attention


# revision 35
# speedup vs baseline: 1.0156x; 1.0156x over previous
"""Trainium2 Bass kernel for batched cross-attention (B=8, Lq=1024, Lk=2048, D=1024).

Sharding: pure data-parallel over the batch dim — each of the 8 NeuronCores
computes full attention for one batch element. Weights are replicated.

Per core:
  Q = q_b @ Wq^T + bq          [1024, 1024]
  K = x_b @ Wk^T + bk          [2048, 1024]
  V = x_b @ Wv^T + bv          [2048, 1024]
  S = Q @ K^T / sqrt(D)        [1024, 2048]
  A = softmax(S, axis=-1)      [1024, 2048]  (output 2)
  O = A @ V                    [1024, 1024]  (output 1)

All matmuls run as float32r (full-rate fp32 on the PE at moving-dim>=256).
The TensorEngine contracts over the partition dim, so every matmul operand
needs the contraction dim (d, e or k) on partitions. x^T, q^T and the W^T
matrices are produced on the HOST (input marshalling in kernel()) and DMA'd
directly in d-major layout — only the two unavoidable on-device transposes
remain (Q -> Q^T after the projection, and exp(S) -> exp(S)^T for the
weighted matmul), done on the PE via identity matmuls with 4 transposes
sharing one PSUM bank per batched PSUM->SBUF copy.

Phase A reads x^T once and computes BOTH K^T (kept in SBUF) and V. V is
bounced through DRAM: its write happens in phase A, its read at the start of
DMA-slack phase B, keeping phase A under the ~200GB/s DMA ceiling.

The weighted matmul consumes the *unnormalized* exp(S) transpose; the softmax
1/rowsum is folded into the PSUM->SBUF copy of the weighted output as a
per-partition scale, so the PE never waits on the softmax reduction.
"""

import sys

import numpy as np

if "/opt/trn_rl_repo" not in sys.path:
    sys.path.insert(0, "/opt/trn_rl_repo")

P = 128
D = 1024  # model dim
LQ = 1024  # query length
LK = 2048  # key length
B = 8  # batch == number of cores
DC = D // P  # 8 chunks of the contraction (d) dim
EC = D // P  # 8 chunks of the output-feature (e) dim
KC = LK // P  # 16 chunks of the key dim
KB = 512  # k-block width for phase A streaming
NKB = LK // KB  # 4
NQB = LQ // P  # 8 query blocks
SCALE = 1.0 / 32.0  # 1/sqrt(D)

_CACHE = {}


def build_nc(use_bias=True):
    import concourse.mybir as mybir
    import concourse.tile as tile
    from concourse import bacc
    from concourse.masks import make_identity

    F32 = mybir.dt.float32
    F32R = mybir.dt.float32r
    Exp = mybir.ActivationFunctionType.Exp
    Ident = mybir.ActivationFunctionType.Identity
    Copy = mybir.ActivationFunctionType.Copy
    AX = mybir.AxisListType.X

    nc = bacc.Bacc(
        "TRN2",
        target_bir_lowering=False,
        debug=False,
        enable_asserts=False,
        num_devices=B,
    )

    # host-transposed (d-major) inputs
    xT_d = nc.dram_tensor("xT", [D, LK], F32R, kind="ExternalInput").ap()
    qT_d = nc.dram_tensor("qT", [D, LQ], F32R, kind="ExternalInput").ap()
    wqT_d = nc.dram_tensor("WqT", [D, D], F32R, kind="ExternalInput").ap()
    wkT_d = nc.dram_tensor("WkT", [D, D], F32R, kind="ExternalInput").ap()
    wvT_d = nc.dram_tensor("WvT", [D, D], F32R, kind="ExternalInput").ap()
    if use_bias:
        bq_d = nc.dram_tensor("bq", [D], F32, kind="ExternalInput").ap()
        bk_d = nc.dram_tensor("bk", [D], F32, kind="ExternalInput").ap()
        bv_d = nc.dram_tensor("bv", [D], F32, kind="ExternalInput").ap()
    out_d = nc.dram_tensor("weighted", [LQ, D], F32, kind="ExternalOutput").ap()
    attn_d = nc.dram_tensor("attention", [LQ, LK], F32, kind="ExternalOutput").ap()
    # V bounce, e-half-major so phase B can fetch each half contiguously:
    # V_dram[eh, p, ko, j] = V[ko*128 + p, eh*512 + j]
    v_dram = nc.dram_tensor("V_scratch", [2, P, KC, 512], F32R).ap()

    # d-major DRAM views: [(c p), n] -> [p, c, n]
    xT_v = xT_d.rearrange("(c p) n -> p c n", p=P)
    qT_v = qT_d.rearrange("(c p) n -> p c n", p=P)
    wqT_v = wqT_d.rearrange("(c p) n -> p c n", p=P)
    wkT_v = wkT_d.rearrange("(c p) n -> p c n", p=P)
    wvT_v = wvT_d.rearrange("(c p) n -> p c n", p=P)

    with tile.TileContext(nc) as tc:
        with (
            tc.tile_pool(name="const", bufs=1) as cpool,
            tc.tile_pool(name="persist", bufs=1) as persist,
            tc.tile_pool(name="psumT4", bufs=3, space="PSUM") as psumT4,
            tc.tile_pool(name="psumMM", bufs=5, space="PSUM") as psumMM,
        ):
            # phase-B working pools live at top level so the A->B handoff has
            # no pool-boundary space dependency
            _pb_ctxs = [
                tc.tile_pool(name="pb_exp", bufs=1),
            ]
            (expp,) = [c.__enter__() for c in _pb_ctxs]

            ident = cpool.tile([P, P], F32, tag="ident")
            make_identity(nc, ident[:])

            if use_bias:
                # per-partition bias layouts: b[(eo p)] -> [p, eo]
                bk_sb = cpool.tile([P, EC], F32, tag="bk")
                nc.sync.dma_start(bk_sb[:], bk_d.rearrange("(o p) -> p o", p=P))
                bq_sb = cpool.tile([P, EC], F32, tag="bq")
                nc.sync.dma_start(bq_sb[:], bq_d.rearrange("(o p) -> p o", p=P))

                # ones-column trick operands for the V bias (free-dim bias):
                # onescol[p, m] = (p == 0); bvpad[0, :] = bv, others zero.
                # psum += onescol.T @ bvpad broadcasts bv to all partitions.
                onescol = cpool.tile([P, P], F32R, tag="onescol")
                bvpad = cpool.tile([P, D], F32R, tag="bvpad")

            # K^T [e, k] and Q^T stay resident from phase A through phase B.
            KT = persist.tile([P, EC, LK], F32R, tag="KT")  # KT[p, eo, k]
            # Q^T in two 4-q-block groups: QTg[g][p, ec, (qb%4)*128 + i]
            QTg = [
                persist.tile([P, EC, 512], F32R, tag=f"QT{g}", name=f"QTg{g}")
                for g in range(2)
            ]

            def transpose_batch(src, src_off, dst4, dst_c0, nblk):
                """PE-transpose `nblk` (<=4) contiguous [P, P] blocks of `src`
                starting at free-offset `src_off`, into dst4[:, dst_c0+j, :]
                via one shared PSUM bank and a single batched copy."""
                pst = psumT4.tile([P, 4, P], F32, tag="pT4")
                for j in range(nblk):
                    nc.tensor.transpose(
                        pst[:, j, :],
                        src[:, src_off + j * P : src_off + (j + 1) * P],
                        ident[:],
                    )
                nc.any.tensor_copy(
                    dst4[:, dst_c0 : dst_c0 + nblk, :], pst[:, :nblk, :]
                )

            # ------------- Phase A: K^T (SBUF) + V (DRAM) from one x^T pass --
            with (
                tc.tile_pool(name="pa_wT", bufs=1) as wTp,
                tc.tile_pool(name="pa_xT", bufs=2) as xTp,
                tc.tile_pool(name="pa_vstg", bufs=2) as vstgp,
                tc.tile_pool(name="pa_stg", bufs=1) as stgp,
            ):
                # PE warmup: dummy fp32 matmuls on the identity so the HAM
                # clock-gate reaches 8/8 while the first DMAs stream.
                wps = psumMM.tile([P, 512], F32, tag="pMM")
                for w in range(8):
                    nc.tensor.matmul(
                        wps[:, 0:P],
                        ident[:],
                        ident[:],
                        start=(w == 0),
                        stop=(w == 7),
                    )

                if use_bias:
                    # f32r tiles can't be memset/DMA'd directly: stage in f32
                    # and let ACT copies do the f32->f32r rounding.
                    stage = stgp.tile([P, D], F32, tag="stg")
                    nc.gpsimd.memset(stage[:], 0.0)
                    nc.gpsimd.memset(stage[0:1, 0:P], 1.0)
                    nc.scalar.copy(onescol[:], stage[:, 0:P])
                    bvstage = stgp.tile([P, D], F32, tag="stg")
                    nc.gpsimd.memset(bvstage[:], 0.0)
                    nc.sync.dma_start(
                        bvstage[0:1, :], bv_d.rearrange("(a d) -> a d", a=1)
                    )
                    nc.scalar.copy(bvpad[:], bvstage[:])

                WkT = wTp.tile([P, DC, D], F32R, tag="WkT")
                WvT = wTp.tile([P, DC, D], F32R, tag="WvT")

                def xt_block(kb, per_dc=False):
                    xT = xTp.tile([P, DC, KB], F32R, tag="xT")
                    if per_dc:
                        # interleave per-dc chunks of WkT and x^T(kb0) so the
                        # first KT matmuls start after ~1MB of DMA, not 6MB
                        for dc in range(DC):
                            nc.sync.dma_start(
                                WkT[:, dc, :], wkT_v[:, dc, :]
                            )
                            nc.sync.dma_start(
                                xT[:, dc, :],
                                xT_v[:, dc, kb * KB : (kb + 1) * KB],
                            )
                        for dc in range(DC):
                            nc.sync.dma_start(
                                WvT[:, dc, :], wvT_v[:, dc, :]
                            )
                    else:
                        for h in range(2):
                            nc.sync.dma_start(
                                xT[:, h * (DC // 2) : (h + 1) * (DC // 2), :],
                                xT_v[
                                    :,
                                    h * (DC // 2) : (h + 1) * (DC // 2),
                                    kb * KB : (kb + 1) * KB,
                                ],
                            )
                    return xT

                def kt_block(kb, xT):
                    # KT[:, ec, kb] += sum_dc WkT[:, dc, ec].T @ xT[:, dc, :]
                    for ec in range(EC):
                        ps = psumMM.tile([P, 512], F32, tag="pMM")
                        for dc in range(DC):
                            nc.tensor.matmul(
                                ps[:, :KB],
                                WkT[:, dc, ec * P : (ec + 1) * P],
                                xT[:, dc, :],
                                start=(dc == 0),
                                stop=(dc == DC - 1),
                            )
                        if use_bias:
                            # copy + per-partition bias bk[e]
                            nc.scalar.activation(
                                KT[:, ec, kb * KB : (kb + 1) * KB],
                                ps[:, :KB],
                                Ident,
                                bias=bk_sb[:, ec : ec + 1],
                            )
                        else:
                            nc.any.tensor_copy(
                                KT[:, ec, kb * KB : (kb + 1) * KB], ps[:, :KB]
                            )

                def v_block(kb, xT):
                    # V[kb*4+k4] = sum_dc xT[:,dc,k4].T @ WvT[:,dc,:] (+ bv),
                    # staged out to DRAM for phase B
                    for k4 in range(KB // P):
                        ko = kb * (KB // P) + k4
                        for eh in range(2):
                            ps = psumMM.tile([P, 512], F32, tag="pMM")
                            if use_bias:
                                nc.tensor.matmul(
                                    ps[:],
                                    onescol[:],
                                    bvpad[:, eh * 512 : (eh + 1) * 512],
                                    start=True,
                                    stop=False,
                                )
                            for dc in range(DC):
                                nc.tensor.matmul(
                                    ps[:],
                                    xT[:, dc, k4 * P : (k4 + 1) * P],
                                    WvT[:, dc, eh * 512 : (eh + 1) * 512],
                                    start=(dc == 0 and not use_bias),
                                    stop=(dc == DC - 1),
                                )
                            vstg = vstgp.tile([P, 512], F32R, tag="vstg")
                            nc.any.tensor_copy(vstg[:], ps[:])
                            nc.sync.dma_start(v_dram[eh, :, ko, :], vstg[:])

                for kb in range(NKB):
                    xT = xt_block(kb, per_dc=(kb == 0))
                    kt_block(kb, xT)
                    v_block(kb, xT)

                # ---- tail of phase A: build Q^T directly (no Q transpose)
                # WqT reuses WkT's slot (same tag); qT reuses the xT slots.
                WqT = wTp.tile([P, DC, D], F32R, tag="WkT")
                nc.sync.dma_start(WqT[:, 0 : DC // 2, :], wqT_v[:, 0 : DC // 2, :])
                nc.sync.dma_start(WqT[:, DC // 2 :, :], wqT_v[:, DC // 2 :, :])
                for g in range(2):
                    qT4 = xTp.tile([P, DC, 512], F32R, tag="xT")
                    nc.sync.dma_start(
                        qT4[:], qT_v[:, :, g * 512 : (g + 1) * 512]
                    )
                    # QT[e, q] = sum_dc WqT[:, dc, e].T @ qT4[:, dc, :] (+ bq)
                    for ec in range(EC):
                        ps = psumMM.tile([P, 512], F32, tag="pMM")
                        for dc in range(DC):
                            nc.tensor.matmul(
                                ps[:],
                                WqT[:, dc, ec * P : (ec + 1) * P],
                                qT4[:, dc, :],
                                start=(dc == 0),
                                stop=(dc == DC - 1),
                            )
                        if use_bias:
                            nc.scalar.activation(
                                QTg[g][:, ec, :],
                                ps[:],
                                Ident,
                                bias=bq_sb[:, ec : ec + 1],
                            )
                        else:
                            nc.any.tensor_copy(QTg[g][:, ec, :], ps[:])

            # ------------- Phase B: per-q-block attention --------------------
            with (
                tc.tile_pool(name="pb_v", bufs=1) as vp,
                tc.tile_pool(name="pb_small", bufs=1) as smallp,
                tc.tile_pool(name="pb_out", bufs=1) as outp,
                tc.tile_pool(name="pb_attnT", bufs=1) as attnTp,
            ):
                # fetch V from the bounce buffer, e-half 0 first (the first
                # weighted matmul needs only half 0)
                V = vp.tile([P, KC, D], F32R, tag="V")  # V[p, ko, e]
                for eh in range(2):
                    nc.sync.dma_start(
                        V[:, :, eh * 512 : (eh + 1) * 512], v_dram[eh]
                    )

                # keep the PE warm across the phase boundary while the V/q
                # DMAs land
                wps = psumMM.tile([P, 512], F32, tag="pMM")
                for w in range(16):
                    nc.tensor.matmul(
                        wps[:, 0:P],
                        ident[:],
                        ident[:],
                        start=(w == 0),
                        stop=(w == 15),
                    )

                for qb in range(NQB):
                    qs = qb * P
                    QT = QTg[qb // 4]
                    qo = (qb % 4) * P

                    # scores (psum) -> exp + row-sum, in chunks of 512
                    exp_sb = expp.tile([P, LK], F32, tag="exp")
                    sums4 = smallp.tile([P, 4], F32, tag="sums4")
                    for kq in range(LK // 512):
                        ps = psumMM.tile([P, 512], F32, tag="pMM")
                        for ec in range(EC):
                            nc.tensor.matmul(
                                ps[:],
                                QT[:, ec, qo : qo + P],
                                KT[:, ec, kq * 512 : (kq + 1) * 512],
                                start=(ec == 0),
                                stop=(ec == EC - 1),
                            )
                        nc.scalar.activation(
                            exp_sb[:, kq * 512 : (kq + 1) * 512],
                            ps[:],
                            Exp,
                            scale=SCALE,
                            accum_out=sums4[:, kq : kq + 1],
                        )

                    # transpose UNNORMALIZED exp -> attnT[k-part, q]; the
                    # 1/rowsum is applied on the weighted output instead,
                    # so the PE never waits on the softmax reduction.
                    attnT = attnTp.tile([P, KC, P], F32R, tag="attnT")
                    for g in range(KC // 4):
                        transpose_batch(exp_sb, g * 4 * P, attnT, g * 4, 4)

                    sumk = smallp.tile([P, 1], F32, tag="sumk")
                    nc.vector.reduce_sum(sumk[:], sums4[:], axis=AX)
                    rsum = smallp.tile([P, 1], F32, tag="rsum")
                    nc.vector.reciprocal(rsum[:], sumk[:])
                    # normalize in place (after the transposes read it),
                    # write attention out
                    nc.vector.tensor_scalar_mul(exp_sb[:], exp_sb[:], rsum[:])
                    nc.sync.dma_start(attn_d[qs : qs + P, :], exp_sb[:])

                    # weighted [q, e] = (sum_kc attnT[:, kc].T @ V) * rsum
                    wout = outp.tile([P, D], F32, tag="wout")
                    for eh in range(2):
                        ps = psumMM.tile([P, 512], F32, tag="pMM")
                        for kc in range(KC):
                            nc.tensor.matmul(
                                ps[:],
                                attnT[:, kc, :],
                                V[:, kc, eh * 512 : (eh + 1) * 512],
                                start=(kc == 0),
                                stop=(kc == KC - 1),
                            )
                        nc.scalar.activation(
                            wout[:, eh * 512 : (eh + 1) * 512],
                            ps[:],
                            Copy,
                            scale=rsum[:],
                        )
                    nc.sync.dma_start(out_d[qs : qs + P, :], wout[:])

            for c in reversed(_pb_ctxs):
                c.__exit__(None, None, None)

    nc.compile()
    return nc


def _get_nc(use_bias=True):
    key = ("nc", use_bias)
    if key not in _CACHE:
        _CACHE[key] = build_nc(use_bias=use_bias)
    return _CACHE[key]


def make_in_maps(inputs, use_bias):
    """Host-side input marshalling: shard over batch and pre-transpose to the
    d-major layouts the kernel consumes."""
    wqT = np.ascontiguousarray(np.asarray(inputs["Wq"], dtype=np.float32).T)
    wkT = np.ascontiguousarray(np.asarray(inputs["Wk"], dtype=np.float32).T)
    wvT = np.ascontiguousarray(np.asarray(inputs["Wv"], dtype=np.float32).T)
    in_maps = []
    for b in range(B):
        m = {
            "xT": np.ascontiguousarray(
                np.asarray(inputs["x"][b], dtype=np.float32).T
            ),
            "qT": np.ascontiguousarray(
                np.asarray(inputs["q"][b], dtype=np.float32).T
            ),
            "WqT": wqT,
            "WkT": wkT,
            "WvT": wvT,
        }
        if use_bias:
            m["bq"] = np.asarray(inputs["bq"], dtype=np.float32)
            m["bk"] = np.asarray(inputs["bk"], dtype=np.float32)
            m["bv"] = np.asarray(inputs["bv"], dtype=np.float32)
        in_maps.append(m)
    return in_maps


def kernel(**inputs):
    from concourse.bass_utils import run_bass_kernel_spmd

    use_bias = any(
        np.any(np.asarray(inputs[k])) for k in ("bq", "bk", "bv")
    )
    nc = _get_nc(use_bias=use_bias)
    in_maps = make_in_maps(inputs, use_bias)
    res = run_bass_kernel_spmd(nc, in_maps, core_ids=list(range(B)))
    weighted = np.stack([res.results[b]["weighted"] for b in range(B)])
    attention = np.stack([res.results[b]["attention"] for b in range(B)])
    return weighted, attention


# revision 36
# speedup vs baseline: 1.0543x; 1.0381x over previous
"""Trainium2 Bass kernel for batched cross-attention (B=8, Lq=1024, Lk=2048, D=1024).

Sharding: pure data-parallel over the batch dim — each of the 8 NeuronCores
computes full attention for one batch element. Weights are replicated.

Per core:
  Q = q_b @ Wq^T + bq          [1024, 1024]
  K = x_b @ Wk^T + bk          [2048, 1024]
  V = x_b @ Wv^T + bv          [2048, 1024]
  S = Q @ K^T / sqrt(D)        [1024, 2048]
  A = softmax(S, axis=-1)      [1024, 2048]  (output 2)
  O = A @ V                    [1024, 1024]  (output 1)

All matmuls run as float32r (full-rate fp32 on the PE at moving-dim>=256).
The TensorEngine contracts over the partition dim, so every matmul operand
needs the contraction dim (d, e or k) on partitions. x^T, q^T and the W^T
matrices are produced on the HOST (input marshalling in kernel()) and DMA'd
directly in d-major layout — only the two unavoidable on-device transposes
remain (Q -> Q^T after the projection, and exp(S) -> exp(S)^T for the
weighted matmul), done on the PE via identity matmuls with 4 transposes
sharing one PSUM bank per batched PSUM->SBUF copy.

Phase A reads x^T once and computes BOTH K^T (kept in SBUF) and V. V is
bounced through DRAM: its write happens in phase A, its read at the start of
DMA-slack phase B, keeping phase A under the ~200GB/s DMA ceiling.

The weighted matmul consumes the *unnormalized* exp(S) transpose; the softmax
1/rowsum is folded into the PSUM->SBUF copy of the weighted output as a
per-partition scale, so the PE never waits on the softmax reduction.
"""

import sys

import numpy as np

if "/opt/trn_rl_repo" not in sys.path:
    sys.path.insert(0, "/opt/trn_rl_repo")

P = 128
D = 1024  # model dim
LQ = 1024  # query length
LK = 2048  # key length
B = 8  # batch == number of cores
DC = D // P  # 8 chunks of the contraction (d) dim
EC = D // P  # 8 chunks of the output-feature (e) dim
KC = LK // P  # 16 chunks of the key dim
KB = 512  # k-block width for phase A streaming
NKB = LK // KB  # 4
NQB = LQ // P  # 8 query blocks
SCALE = 1.0 / 32.0  # 1/sqrt(D)

_CACHE = {}


def build_nc(use_bias=True):
    import concourse.mybir as mybir
    import concourse.tile as tile
    from concourse import bacc
    from concourse.masks import make_identity

    F32 = mybir.dt.float32
    F32R = mybir.dt.float32r
    Exp = mybir.ActivationFunctionType.Exp
    Ident = mybir.ActivationFunctionType.Identity
    Copy = mybir.ActivationFunctionType.Copy
    AX = mybir.AxisListType.X

    nc = bacc.Bacc(
        "TRN2",
        target_bir_lowering=False,
        debug=False,
        enable_asserts=False,
        num_devices=B,
    )

    # host-transposed (d-major) inputs
    xT_d = nc.dram_tensor("xT", [D, LK], F32R, kind="ExternalInput").ap()
    qT_d = nc.dram_tensor("qT", [D, LQ], F32R, kind="ExternalInput").ap()
    wqT_d = nc.dram_tensor("WqT", [D, D], F32R, kind="ExternalInput").ap()
    wkT_d = nc.dram_tensor("WkT", [D, D], F32R, kind="ExternalInput").ap()
    wvT_d = nc.dram_tensor("WvT", [D, D], F32R, kind="ExternalInput").ap()
    if use_bias:
        bq_d = nc.dram_tensor("bq", [D], F32, kind="ExternalInput").ap()
        bk_d = nc.dram_tensor("bk", [D], F32, kind="ExternalInput").ap()
        bv_d = nc.dram_tensor("bv", [D], F32, kind="ExternalInput").ap()
    out_d = nc.dram_tensor("weighted", [LQ, D], F32, kind="ExternalOutput").ap()
    attn_d = nc.dram_tensor("attention", [LQ, LK], F32, kind="ExternalOutput").ap()
    # V bounce, e-half-major so phase B can fetch each half contiguously:
    # V_dram[eh, p, ko, j] = V[ko*128 + p, eh*512 + j]
    v_dram = nc.dram_tensor("V_scratch", [2, P, KC, 512], F32R).ap()

    # d-major DRAM views: [(c p), n] -> [p, c, n]
    xT_v = xT_d.rearrange("(c p) n -> p c n", p=P)
    qT_v = qT_d.rearrange("(c p) n -> p c n", p=P)
    wqT_v = wqT_d.rearrange("(c p) n -> p c n", p=P)
    wkT_v = wkT_d.rearrange("(c p) n -> p c n", p=P)
    wvT_v = wvT_d.rearrange("(c p) n -> p c n", p=P)

    with tile.TileContext(nc) as tc:
        with (
            tc.tile_pool(name="const", bufs=1) as cpool,
            tc.tile_pool(name="persist", bufs=1) as persist,
            tc.tile_pool(name="psumT4", bufs=3, space="PSUM") as psumT4,
            tc.tile_pool(name="psumMM", bufs=5, space="PSUM") as psumMM,
        ):
            ident = cpool.tile([P, P], F32, tag="ident")
            make_identity(nc, ident[:])

            if use_bias:
                # per-partition bias layouts: b[(eo p)] -> [p, eo]
                bk_sb = cpool.tile([P, EC], F32, tag="bk")
                nc.sync.dma_start(bk_sb[:], bk_d.rearrange("(o p) -> p o", p=P))
                bq_sb = cpool.tile([P, EC], F32, tag="bq")
                nc.sync.dma_start(bq_sb[:], bq_d.rearrange("(o p) -> p o", p=P))

                # ones-column trick operands for the V bias (free-dim bias):
                # onescol[p, m] = (p == 0); bvpad[0, :] = bv, others zero.
                # psum += onescol.T @ bvpad broadcasts bv to all partitions.
                onescol = cpool.tile([P, P], F32R, tag="onescol")
                bvpad = cpool.tile([P, D], F32R, tag="bvpad")

            # K^T [e, k] and Q^T stay resident from phase A through phase B.
            KT = persist.tile([P, EC, LK], F32R, tag="KT")  # KT[p, eo, k]
            # Q^T in two 4-q-block groups: QTg[g][p, ec, (qb%4)*128 + i]
            QTg = [
                persist.tile([P, EC, 512], F32R, tag=f"QT{g}", name=f"QTg{g}")
                for g in range(2)
            ]

            def transpose_batch(src, src_off, dst4, dst_c0, nblk):
                """PE-transpose `nblk` (<=4) contiguous [P, P] blocks of `src`
                starting at free-offset `src_off`, into dst4[:, dst_c0+j, :]
                via one shared PSUM bank and a single batched copy."""
                pst = psumT4.tile([P, 4, P], F32, tag="pT4")
                for j in range(nblk):
                    nc.tensor.transpose(
                        pst[:, j, :],
                        src[:, src_off + j * P : src_off + (j + 1) * P],
                        ident[:],
                    )
                nc.any.tensor_copy(
                    dst4[:, dst_c0 : dst_c0 + nblk, :], pst[:, :nblk, :]
                )

            # ------------- Phase A: K^T (SBUF) + V (DRAM) from one x^T pass --
            with (
                tc.tile_pool(name="pa_wT", bufs=1) as wTp,
                tc.tile_pool(name="pa_xT", bufs=2) as xTp,
                tc.tile_pool(name="pa_vstg", bufs=2) as vstgp,
                tc.tile_pool(name="pa_stg", bufs=1) as stgp,
            ):
                # PE warmup: dummy fp32 matmuls on the identity so the HAM
                # clock-gate reaches 8/8 while the first DMAs stream.
                wps = psumMM.tile([P, 512], F32, tag="pMM")
                for w in range(8):
                    nc.tensor.matmul(
                        wps[:, 0:P],
                        ident[:],
                        ident[:],
                        start=(w == 0),
                        stop=(w == 7),
                    )

                if use_bias:
                    # f32r tiles can't be memset/DMA'd directly: stage in f32
                    # and let ACT copies do the f32->f32r rounding.
                    stage = stgp.tile([P, D], F32, tag="stg")
                    nc.gpsimd.memset(stage[:], 0.0)
                    nc.gpsimd.memset(stage[0:1, 0:P], 1.0)
                    nc.scalar.copy(onescol[:], stage[:, 0:P])
                    bvstage = stgp.tile([P, D], F32, tag="stg")
                    nc.gpsimd.memset(bvstage[:], 0.0)
                    nc.sync.dma_start(
                        bvstage[0:1, :], bv_d.rearrange("(a d) -> a d", a=1)
                    )
                    nc.scalar.copy(bvpad[:], bvstage[:])

                WkT = wTp.tile([P, DC, D], F32R, tag="WkT")
                WvT = wTp.tile([P, DC, D], F32R, tag="WvT")

                def xt_block(kb, per_dc=False):
                    xT = xTp.tile([P, DC, KB], F32R, tag="xT")
                    if per_dc:
                        # interleave per-dc chunks of WkT and x^T(kb0) so the
                        # first KT matmuls start after ~1MB of DMA, not 6MB
                        for dc in range(DC):
                            nc.sync.dma_start(
                                WkT[:, dc, :], wkT_v[:, dc, :]
                            )
                            nc.sync.dma_start(
                                xT[:, dc, :],
                                xT_v[:, dc, kb * KB : (kb + 1) * KB],
                            )
                        for dc in range(DC):
                            nc.sync.dma_start(
                                WvT[:, dc, :], wvT_v[:, dc, :]
                            )
                    else:
                        for h in range(2):
                            nc.sync.dma_start(
                                xT[:, h * (DC // 2) : (h + 1) * (DC // 2), :],
                                xT_v[
                                    :,
                                    h * (DC // 2) : (h + 1) * (DC // 2),
                                    kb * KB : (kb + 1) * KB,
                                ],
                            )
                    return xT

                def kt_block(kb, xT):
                    # KT[:, ec, kb] += sum_dc WkT[:, dc, ec].T @ xT[:, dc, :]
                    for ec in range(EC):
                        ps = psumMM.tile([P, 512], F32, tag="pMM")
                        for dc in range(DC):
                            nc.tensor.matmul(
                                ps[:, :KB],
                                WkT[:, dc, ec * P : (ec + 1) * P],
                                xT[:, dc, :],
                                start=(dc == 0),
                                stop=(dc == DC - 1),
                            )
                        if use_bias:
                            # copy + per-partition bias bk[e]
                            nc.scalar.activation(
                                KT[:, ec, kb * KB : (kb + 1) * KB],
                                ps[:, :KB],
                                Ident,
                                bias=bk_sb[:, ec : ec + 1],
                            )
                        else:
                            nc.any.tensor_copy(
                                KT[:, ec, kb * KB : (kb + 1) * KB], ps[:, :KB]
                            )

                def v_block(kb, xT):
                    # V[kb*4+k4] = sum_dc xT[:,dc,k4].T @ WvT[:,dc,:] (+ bv),
                    # staged out to DRAM for phase B
                    for k4 in range(KB // P):
                        ko = kb * (KB // P) + k4
                        for eh in range(2):
                            ps = psumMM.tile([P, 512], F32, tag="pMM")
                            if use_bias:
                                nc.tensor.matmul(
                                    ps[:],
                                    onescol[:],
                                    bvpad[:, eh * 512 : (eh + 1) * 512],
                                    start=True,
                                    stop=False,
                                )
                            for dc in range(DC):
                                nc.tensor.matmul(
                                    ps[:],
                                    xT[:, dc, k4 * P : (k4 + 1) * P],
                                    WvT[:, dc, eh * 512 : (eh + 1) * 512],
                                    start=(dc == 0 and not use_bias),
                                    stop=(dc == DC - 1),
                                )
                            vstg = vstgp.tile([P, 512], F32R, tag="vstg")
                            nc.any.tensor_copy(vstg[:], ps[:])
                            nc.sync.dma_start(v_dram[eh, :, ko, :], vstg[:])

                for kb in range(NKB):
                    xT = xt_block(kb, per_dc=(kb == 0))
                    kt_block(kb, xT)
                    v_block(kb, xT)

                # ---- tail of phase A: build Q^T directly (no Q transpose)
                # WqT reuses WkT's slot (same tag); qT reuses the xT slots.
                WqT = wTp.tile([P, DC, D], F32R, tag="WkT")
                nc.sync.dma_start(WqT[:, 0 : DC // 2, :], wqT_v[:, 0 : DC // 2, :])
                nc.sync.dma_start(WqT[:, DC // 2 :, :], wqT_v[:, DC // 2 :, :])
                for g in range(2):
                    qT4 = xTp.tile([P, DC, 512], F32R, tag="xT")
                    nc.sync.dma_start(
                        qT4[:], qT_v[:, :, g * 512 : (g + 1) * 512]
                    )
                    # QT[e, q] = sum_dc WqT[:, dc, e].T @ qT4[:, dc, :] (+ bq)
                    for ec in range(EC):
                        ps = psumMM.tile([P, 512], F32, tag="pMM")
                        for dc in range(DC):
                            nc.tensor.matmul(
                                ps[:],
                                WqT[:, dc, ec * P : (ec + 1) * P],
                                qT4[:, dc, :],
                                start=(dc == 0),
                                stop=(dc == DC - 1),
                            )
                        if use_bias:
                            nc.scalar.activation(
                                QTg[g][:, ec, :],
                                ps[:],
                                Ident,
                                bias=bq_sb[:, ec : ec + 1],
                            )
                        else:
                            nc.any.tensor_copy(QTg[g][:, ec, :], ps[:])

            # ------------- Phase B: per-q-block attention --------------------
            with (
                tc.tile_pool(name="pb_v", bufs=1) as vp,
                tc.tile_pool(name="pb_small", bufs=2) as smallp,
                tc.tile_pool(name="pb_out", bufs=1) as outp,
                tc.tile_pool(name="pb_attnT", bufs=1) as attnTp,
                tc.tile_pool(name="pb_exp", bufs=2) as expp,
            ):
                # fetch V from the bounce buffer, e-half 0 first (the first
                # weighted matmul needs only half 0)
                V = vp.tile([P, KC, D], F32R, tag="V")  # V[p, ko, e]
                for eh in range(2):
                    nc.sync.dma_start(
                        V[:, :, eh * 512 : (eh + 1) * 512], v_dram[eh]
                    )

                # keep the PE warm across the phase boundary while the V/q
                # DMAs land
                wps = psumMM.tile([P, 512], F32, tag="pMM")
                for w in range(16):
                    nc.tensor.matmul(
                        wps[:, 0:P],
                        ident[:],
                        ident[:],
                        start=(w == 0),
                        stop=(w == 15),
                    )

                def scores_exp(qb):
                    # scores (psum) -> exp + row-sum, in chunks of 512
                    QT = QTg[qb // 4]
                    qo = (qb % 4) * P
                    exp_sb = expp.tile([P, LK], F32, tag="exp")
                    sums4 = smallp.tile([P, 4], F32, tag="sums4")
                    for kq in range(LK // 512):
                        ps = psumMM.tile([P, 512], F32, tag="pMM")
                        for ec in range(EC):
                            nc.tensor.matmul(
                                ps[:],
                                QT[:, ec, qo : qo + P],
                                KT[:, ec, kq * 512 : (kq + 1) * 512],
                                start=(ec == 0),
                                stop=(ec == EC - 1),
                            )
                        nc.scalar.activation(
                            exp_sb[:, kq * 512 : (kq + 1) * 512],
                            ps[:],
                            Exp,
                            scale=SCALE,
                            accum_out=sums4[:, kq : kq + 1],
                        )
                    return exp_sb, sums4

                def finish_block(qb, exp_sb, sums4):
                    qs = qb * P
                    # transpose UNNORMALIZED exp -> attnT[k-part, q]; the
                    # 1/rowsum is applied on the weighted output instead,
                    # so the PE never waits on the softmax reduction.
                    attnT = attnTp.tile([P, KC, P], F32R, tag="attnT")
                    for g in range(KC // 4):
                        transpose_batch(exp_sb, g * 4 * P, attnT, g * 4, 4)

                    sumk = smallp.tile([P, 1], F32, tag="sumk")
                    nc.vector.reduce_sum(sumk[:], sums4[:], axis=AX)
                    rsum = smallp.tile([P, 1], F32, tag="rsum")
                    nc.vector.reciprocal(rsum[:], sumk[:])
                    # normalize in place (after the transposes read it),
                    # write attention out
                    nc.vector.tensor_scalar_mul(exp_sb[:], exp_sb[:], rsum[:])
                    nc.sync.dma_start(attn_d[qs : qs + P, :], exp_sb[:])

                    # weighted [q, e] = (sum_kc attnT[:, kc].T @ V) * rsum
                    wout = outp.tile([P, D], F32, tag="wout")
                    for eh in range(2):
                        ps = psumMM.tile([P, 512], F32, tag="pMM")
                        for kc in range(KC):
                            nc.tensor.matmul(
                                ps[:],
                                attnT[:, kc, :],
                                V[:, kc, eh * 512 : (eh + 1) * 512],
                                start=(kc == 0),
                                stop=(kc == KC - 1),
                            )
                        nc.scalar.activation(
                            wout[:, eh * 512 : (eh + 1) * 512],
                            ps[:],
                            Copy,
                            scale=rsum[:],
                        )
                    nc.sync.dma_start(out_d[qs : qs + P, :], wout[:])

                # software pipeline: block qb's attnT/weighted overlap block
                # qb+1's scores, so the PE never waits on the exp tail
                prev = None
                for qb in range(NQB):
                    cur = scores_exp(qb)
                    if prev is not None:
                        finish_block(qb - 1, *prev)
                    prev = cur
                finish_block(NQB - 1, *prev)

    nc.compile()
    return nc


def _get_nc(use_bias=True):
    key = ("nc", use_bias)
    if key not in _CACHE:
        _CACHE[key] = build_nc(use_bias=use_bias)
    return _CACHE[key]


def make_in_maps(inputs, use_bias):
    """Host-side input marshalling: shard over batch and pre-transpose to the
    d-major layouts the kernel consumes."""
    wqT = np.ascontiguousarray(np.asarray(inputs["Wq"], dtype=np.float32).T)
    wkT = np.ascontiguousarray(np.asarray(inputs["Wk"], dtype=np.float32).T)
    wvT = np.ascontiguousarray(np.asarray(inputs["Wv"], dtype=np.float32).T)
    in_maps = []
    for b in range(B):
        m = {
            "xT": np.ascontiguousarray(
                np.asarray(inputs["x"][b], dtype=np.float32).T
            ),
            "qT": np.ascontiguousarray(
                np.asarray(inputs["q"][b], dtype=np.float32).T
            ),
            "WqT": wqT,
            "WkT": wkT,
            "WvT": wvT,
        }
        if use_bias:
            m["bq"] = np.asarray(inputs["bq"], dtype=np.float32)
            m["bk"] = np.asarray(inputs["bk"], dtype=np.float32)
            m["bv"] = np.asarray(inputs["bv"], dtype=np.float32)
        in_maps.append(m)
    return in_maps


def kernel(**inputs):
    from concourse.bass_utils import run_bass_kernel_spmd

    use_bias = any(
        np.any(np.asarray(inputs[k])) for k in ("bq", "bk", "bv")
    )
    nc = _get_nc(use_bias=use_bias)
    in_maps = make_in_maps(inputs, use_bias)
    res = run_bass_kernel_spmd(nc, in_maps, core_ids=list(range(B)))
    weighted = np.stack([res.results[b]["weighted"] for b in range(B)])
    attention = np.stack([res.results[b]["attention"] for b in range(B)])
    return weighted, attention
